# revision 1
# baseline (speedup 1.0000x reference)
"""Trainium2 Bass kernel for nn_Attention_2 (8-head attention with positional bias).

Sharding: one head per NeuronCore (8 heads / 8 cores), data-parallel over the
full batch within each core.  Each core computes its head's projections,
attention (unnormalized softmax via exp(sim)*exp(bias) with the denominator
accumulated through a ones-column in V), and its partial contribution to the
output projection.  The host sums the 8 partial outputs.

Device layout notes:
- q/k/v are sent transposed ([d, n] per batch), bf16, and pre-tiled so every
  DMA is a single contiguous block.
- qh/kh are produced partition-replicated x2 (weights replicated on the host),
  so the K=32 sim matmuls run pairwise-concurrent in disjoint PE row groups.
- exp(pos_bias) is precomputed on the host (exp(sim+bias) = exp(sim)*exp(bias)),
  transposed to [j, i], tiled to match the sim PSUM layout, and sent in bf16.
- All matmul accumulation and the softmax denominator/normalization stay fp32;
  bf16 is used for matmul operands and the post-exp elementwise path.
- VAR="pair2": attn@v accumulators for batch pairs are packed into PE column
  groups 0-63 / 64-127 of a shared PSUM bank and their matmuls emitted
  interleaved, so the two streams (different moving operands) run concurrently
  on separate XBUSes; likewise the q/k projections share one PSUM tile with q
  in columns 0-63 and k in 64-127; VAR="pair4" additionally round-robins the
  output tail across batches (all po->SBUF copies, then transposes+reciprocals,
  then out-proj matmuls interleaved with scale/store) so DVE and PE pipeline at
  i-block boundaries.  Measured 238.1us vs 250-254us for the unpaired schedule (the fp32 output-projection tail is deliberately kept:
  its low-duty matmuls let the PE activity governor grant full clock to the
  dense attention streams).
"""

import sys

sys.path.insert(0, "/opt/trn_rl_repo")

import numpy as np
import ml_dtypes
from contextlib import ExitStack

import concourse.bass as bass  # noqa: F401
import concourse.tile as tile
from concourse import bacc, mybir
from concourse.bass_utils import run_bass_kernel_spmd

B, N, D, H, DH = 4, 2048, 256, 8, 32
SCALE = DH ** -0.5
NCORES = 8
IB = 512            # i-block (query columns per matmul)
NIB = N // IB       # 4
JC = 128            # j-chunk (key rows per partition tile)
NJC = N // JC       # 16
JP = 2              # j-chunks packed per PE pass (row groups)
NJG = NJC // JP     # 8
F32 = mybir.dt.float32
BF16 = mybir.dt.bfloat16
AF = mybir.ActivationFunctionType

QK_BF16 = True      # q/k/v inputs, projection weights, qh/kh in bf16
VAR = "pair4"       # kernel variant used by kernel() / make_in_maps


def build_kernel(nc, qT, kT, vT, wq, wk, wv, wo, eb, out, reps=1, var="base"):
    emit = _emit_body_v2 if var.startswith("v2") else (
        lambda *a: _emit_body(*a, var))
    with tile.TileContext(nc) as tc:
        if reps == 1:
            emit(nc, tc, qT, kT, vT, wq, wk, wv, wo, eb, out)
        else:
            with tc.For_i(0, reps, 1):
                emit(nc, tc, qT, kT, vT, wq, wk, wv, wo, eb, out)


def _emit_body(nc, tc, qT, kT, vT, wq, wk, wv, wo, eb, out, var="base"):
    QKD = BF16 if QK_BF16 else F32
    with ExitStack() as ctx:
        consts = ctx.enter_context(tc.tile_pool(name="consts", bufs=1))
        persist = ctx.enter_context(tc.tile_pool(name="persist", bufs=1))
        qkv_pool = ctx.enter_context(tc.tile_pool(name="qkv", bufs=4))
        work = ctx.enter_context(tc.tile_pool(name="work", bufs=12 if var == "wb12" else 8))
        outp = ctx.enter_context(tc.tile_pool(name="outp", bufs=4))
        psS = ctx.enter_context(tc.tile_pool(name="psS", bufs=3 if var == "bout" else 2, space="PSUM"))
        psO = ctx.enter_context(tc.tile_pool(name="psO", bufs=2 if var == "bout" else 4, space="PSUM"))

        wq_sb = consts.tile([128, 2, JP * DH], QKD)
        nc.sync.dma_start(wq_sb[:], wq[:, :, :])
        wk_sb = consts.tile([128, 2, JP * DH], QKD)
        nc.sync.dma_start(wk_sb[:], wk[:, :, :])
        wv_sb = consts.tile([128, 2, DH], QKD)
        nc.sync.dma_start(wv_sb[:], wv[:, :, :])
        if var == "b16p":
            wo_sb = consts.tile([128, D], BF16)
        elif var in ("pair", "pair2", "pair3", "pair4", "pair5"):
            wo_sb = consts.tile([128, D], F32)
        else:
            wo_sb = consts.tile([DH, D], BF16 if var == "b16" else F32)
        nc.sync.dma_start(wo_sb[:], wo[:, :])
        ident = consts.tile([128, 1], F32)
        nc.vector.memset(ident[:], 1.0)
        identb = consts.tile([128, 1], BF16)
        nc.vector.memset(identb[:], 1.0)

        qh = [persist.tile([JP * DH, N], QKD, name=f"qh{b}") for b in range(B)]
        kh = [persist.tile([JP * DH, N], QKD, name=f"kh{b}") for b in range(B)]
        vh = [persist.tile([128, NJC, DH + 1], BF16, name=f"vh{b}") for b in range(B)]

        # ---- projections (one [*, ib] chunk of all batches) ----
        def emit_proj_chunk(ib):
            qdma = nc.scalar if var == "qsc" else nc.sync
            isl = slice(ib * IB, (ib + 1) * IB)
            for b in range(B):
                qt = qkv_pool.tile([128, 2, IB], QKD, name="qt")
                qdma.dma_start(qt[:], qT[b, ib])
                if var in ("pair2", "pair3", "pair4", "pair5"):
                    # q in PE columns 0-63, k in 64-127: different moving
                    # operands stream concurrently on separate XBUSes
                    kt = qkv_pool.tile([128, 2, IB], QKD, name="kt")
                    qdma.dma_start(kt[:], kT[b, ib])
                    psqk = psS.tile([128, IB], F32, name="psqk", tag="s")
                    nc.tensor.matmul(psqk[0:64, :], wq_sb[:, 0, :], qt[:, 0, :],
                                     start=True, stop=False, skip_group_check=True)
                    nc.tensor.matmul(psqk[64:128, :], wk_sb[:, 0, :], kt[:, 0, :],
                                     start=True, stop=False, skip_group_check=True)
                    nc.tensor.matmul(psqk[0:64, :], wq_sb[:, 1, :], qt[:, 1, :],
                                     start=False, stop=True, skip_group_check=True)
                    nc.tensor.matmul(psqk[64:128, :], wk_sb[:, 1, :], kt[:, 1, :],
                                     start=False, stop=True, skip_group_check=True)
                    nc.vector.tensor_copy(qh[b][:, isl], psqk[0:64, :])
                    nc.vector.tensor_copy(kh[b][:, isl], psqk[64:128, :])
                else:
                    psq = psS.tile([JP * DH, IB], F32, name="psq", tag="s")
                    nc.tensor.matmul(psq[:], wq_sb[:, 0, :], qt[:, 0, :], start=True, stop=False)
                    nc.tensor.matmul(psq[:], wq_sb[:, 1, :], qt[:, 1, :], start=False, stop=True)
                    nc.vector.tensor_copy(qh[b][:, isl], psq[:])

                    kt = qkv_pool.tile([128, 2, IB], QKD, name="kt")
                    qdma.dma_start(kt[:], kT[b, ib])
                    psk = psS.tile([JP * DH, IB], F32, name="psk", tag="s")
                    nc.tensor.matmul(psk[:], wk_sb[:, 0, :], kt[:, 0, :], start=True, stop=False)
                    nc.tensor.matmul(psk[:], wk_sb[:, 1, :], kt[:, 1, :], start=False, stop=True)
                    nc.vector.tensor_copy(kh[b][:, isl], psk[:])

                vt = qkv_pool.tile([128, 2, IB], QKD, name="vt")
                qdma.dma_start(vt[:], vT[b, ib])
                psv = psS.tile([128, IB // JC, DH], F32, name="psv", tag="s")
                for jl in range(IB // JC):
                    jsl = slice(jl * JC, (jl + 1) * JC)
                    nc.tensor.matmul(psv[:, jl, :], vt[:, 0, jsl], wv_sb[:, 0, :],
                                     start=True, stop=False, skip_group_check=True)
                    nc.tensor.matmul(psv[:, jl, :], vt[:, 1, jsl], wv_sb[:, 1, :],
                                     start=False, stop=True, skip_group_check=True)
                nc.vector.tensor_copy(vh[b][:, ib * (IB // JC) : (ib + 1) * (IB // JC), 0:DH], psv[:])

        for b in range(B):
            nc.vector.memset(vh[b][:, :, DH : DH + 1], 1.0)

        # ---- attention + output projection ----
        def emit_attnv(unit):
            b, jg, wt = unit
            for t in range(JP):
                jc = jg * JP + t
                nc.tensor.matmul(
                    po[b][:], vh[b][:, jc, :], wt[:, t, :],
                    start=(jc == 0), stop=(jc == NJC - 1),
                )

        state = {"pending": None, "hpend": []}

        def po_ap(po_b, rows=DH + 1):
            if isinstance(po_b, tuple):
                t_, pr = po_b
                return t_[pr : pr + rows, :]
            return po_b[:]

        def emit_attnv2(entry):
            _ib, po_b, b, jg, wt = entry
            for t in range(JP):
                jc = jg * JP + t
                nc.tensor.matmul(
                    po_ap(po_b), vh[b][:, jc, :], wt[:, t, :],
                    start=(jc == 0), stop=(jc == NJC - 1),
                    skip_group_check=isinstance(po_b, tuple),
                )

        def emit_attnv2_pair(e0, e1):
            # two accumulators in PE column groups 0-63 / 64-127: interleave
            # so both streams run concurrently on separate XBUSes
            _ib0, po_b0, b0, jg0, wt0 = e0
            _ib1, po_b1, b1, jg1, wt1 = e1
            ok = (isinstance(po_b0, tuple) and isinstance(po_b1, tuple)
                  and jg0 == jg1 and po_b0[0] is po_b1[0] and po_b0[1] != po_b1[1])
            if not ok:
                emit_attnv2(e0)
                emit_attnv2(e1)
                return
            for t in range(JP):
                jc = jg0 * JP + t
                nc.tensor.matmul(
                    po_ap(po_b0), vh[b0][:, jc, :], wt0[:, t, :],
                    start=(jc == 0), stop=(jc == NJC - 1), skip_group_check=True,
                )
                nc.tensor.matmul(
                    po_ap(po_b1), vh[b1][:, jc, :], wt1[:, t, :],
                    start=(jc == 0), stop=(jc == NJC - 1), skip_group_check=True,
                )

        def flush_fifo(ib_match=None):
            keep = []
            pend = []
            for e in state.get("fifo", []):
                if ib_match is None or e[0] == ib_match:
                    pend.append(e)
                else:
                    keep.append(e)
            while len(pend) >= 2:
                emit_attnv2_pair(pend.pop(0), pend.pop(0))
            for e in pend:
                emit_attnv2(e)
            state["fifo"] = keep

        ebts = {}

        def get_ebt(ib_, jg_):
            if (ib_, jg_) not in ebts:
                t_ = work.tile([128, JP, IB], BF16, name="ebt", tag="ebt", bufs=12)
                nc.sync.dma_start(t_[:], eb[ib_, jg_])
                ebts[(ib_, jg_)] = t_
            return ebts[(ib_, jg_)]

        def emit_jg2(ib, jg, po_l, active=None):
            if active is None:
                active = ib
            # software-pipeline attn@v late so the in-order PE stream never
            # waits on the DVE multiply; fifo entries carry their accumulator
            isl = slice(ib * IB, (ib + 1) * IB)
            if var == "pair3":
                ebt = get_ebt(ib, jg)
                nxt = (ib, jg + 1) if jg + 1 < NJG else (ib + 1, 0)
                if nxt[0] < NIB:
                    get_ebt(*nxt)
            else:
                ebt = work.tile([128, JP, IB], BF16, name="ebt", tag="ebt", bufs=12)
                nc.sync.dma_start(ebt[:], eb[ib, jg])
            for b in range(B):
                ps = psS.tile([128, JP, IB], F32, name="ps", tag="s")
                for t in range(JP):
                    jc = jg * JP + t
                    psl = slice(32 * t, 32 * (t + 1))
                    nc.tensor.matmul(
                        ps[:, t, :],
                        kh[b][psl, jc * JC : (jc + 1) * JC],
                        qh[b][psl, isl],
                        start=True, stop=True,
                    )
                es = work.tile([128, JP, IB], BF16, name="es")
                nc.scalar.activation(es[:], ps[:], AF.Exp)
                wt = work.tile([128, JP, IB], BF16, name="wt", tag="wt", bufs=20)
                meng = nc.vector
                meng.tensor_mul(wt[:], es[:], ebt[:])
                state.setdefault("fifo", [])
                state["fifo"].append((ib, po_l[b], b, jg, wt))
                if var in ("b16p", "pair", "pair2", "pair3", "pair4", "pair5"):
                    lag = {"pair3": 6, "pair5": 2}.get(var, 4)
                    while (len(state["fifo"]) > lag
                           and state["fifo"][0][0] == active
                           and state["fifo"][1][0] == active):
                        emit_attnv2_pair(state["fifo"].pop(0), state["fifo"].pop(0))
                else:
                    while len(state["fifo"]) > 2 and state["fifo"][0][0] == active:
                        emit_attnv2(state["fifo"].pop(0))

        def emit_jg(ib, jg):
            emit_jg2(ib, jg, po)

        def emit_unit(ib, b, jg, ebt):
            isl = slice(ib * IB, (ib + 1) * IB)
            ps = psS.tile([128, JP, IB], F32, name="ps", tag="s")
            for t in range(JP):
                jc = jg * JP + t
                psl = slice(32 * t, 32 * (t + 1))
                nc.tensor.matmul(
                    ps[:, t, :],
                    kh[b][psl, jc * JC : (jc + 1) * JC],
                    qh[b][psl, isl],
                    start=True, stop=True,
                )
            es = work.tile([128, JP, IB], BF16, name="es")
            nc.scalar.activation(es[:], ps[:], AF.Exp)
            wt = work.tile([128, JP, IB], BF16, name="wt")
            nc.vector.tensor_mul(wt[:], es[:], ebt[:])
            if state["pending"] is not None:
                emit_attnv(state["pending"])
            state["pending"] = (b, jg, wt)

        def emit_attnv_half(h):
            b, jc, wt_t = h
            nc.tensor.matmul(
                po[b][:], vh[b][:, jc, :], wt_t[:],
                start=(jc == 0), stop=(jc == NJC - 1),
            )

        def emit_jg_esplit(ib, jg):
            isl = slice(ib * IB, (ib + 1) * IB)
            ebt = work.tile([128, JP, IB], BF16, name="ebt")
            nc.sync.dma_start(ebt[:], eb[ib, jg])
            for b in range(B):
                ps = psS.tile([128, JP, IB], F32, name="ps", tag="s")
                for t in range(JP):
                    jc = jg * JP + t
                    psl = slice(32 * t, 32 * (t + 1))
                    nc.tensor.matmul(
                        ps[:, t, :],
                        kh[b][psl, jc * JC : (jc + 1) * JC],
                        qh[b][psl, isl],
                        start=True, stop=True,
                    )
                for t in range(JP):
                    jc = jg * JP + t
                    es_t = work.tile([128, IB], BF16, name="esh")
                    nc.scalar.activation(es_t[:], ps[:, t, :], AF.Exp)
                    wt_t = work.tile([128, IB], BF16, name="wth")
                    nc.vector.tensor_mul(wt_t[:], es_t[:], ebt[:, t, :])
                    if state["hpend"]:
                        emit_attnv_half(state["hpend"].pop(0))
                    state["hpend"].append((b, jc, wt_t))

        OTD = BF16 if var in ("b16", "b16p") else F32
        idt = identb if var in ("b16", "b16p") else ident

        ZS = 2 if OTD is BF16 else 1   # pad bf16 PSUM columns to 4-byte alignment

        def emit_tail_b(ib, b, po_b):
            ot = outp.tile([DH + 1, IB], OTD, name="ot")
            nc.vector.tensor_copy(ot[:], po_b[:])
            zt = psO.tile([128, ZS * NIB], OTD, name="zt", tag="po")
            for t_ in range(IB // JC):
                nc.tensor.matmul(
                    zt[:, ZS * t_ : ZS * t_ + 1],
                    ot[DH : DH + 1, t_ * JC : (t_ + 1) * JC],
                    idt[DH : DH + 1, :],
                    is_transpose=True,
                    skip_group_check=True,
                )
            rt = outp.tile([128, ZS * NIB], F32, name="rt")
            nc.vector.reciprocal(rt[:], zt[:])
            for t_ in range(IB // JC):
                pw = psO.tile([128, D], F32, name="pw", tag="po")
                nc.tensor.matmul(pw[:], ot[0:DH, t_ * JC : (t_ + 1) * JC], wo_sb[:], start=True, stop=True)
                ft = outp.tile([128, D], OTD, name="ft")
                nc.vector.tensor_scalar_mul(ft[:], pw[:], rt[:, ZS * t_ : ZS * t_ + 1])
                r0 = ib * IB + t_ * JC
                nc.sync.dma_start(out[b, r0 : r0 + JC, :], ft[:])

        if var == "bout":
            # batch-outer attention: one attn@v accumulator live at a time
            # (2 po banks double-buffered), 3 sim-psum slots, eb tiles cached
            # in SBUF across the 4 batch sweeps of each i-block
            for ib in range(NIB):
                ebts = [None] * NJG

                def get_ebt(jg, _ebts=ebts, _ib=ib):
                    if _ebts[jg] is None:
                        t = work.tile([128, JP, IB], BF16, name="ebt", tag="ebtc", bufs=16)
                        nc.sync.dma_start(t[:], eb[_ib, jg])
                        _ebts[jg] = t
                    return _ebts[jg]

                for b in range(B):
                    po_b = psO.tile([DH + 1, IB], F32, name="pob", tag="po")
                    po = [po_b] * B  # emit_attnv indexes po[b]
                    if ib == 0 and b == 0:
                        for ibk in range(NIB):
                            emit_proj_chunk(ibk)
                            emit_unit(0, 0, 2 * ibk, get_ebt(2 * ibk))
                            emit_unit(0, 0, 2 * ibk + 1, get_ebt(2 * ibk + 1))
                    else:
                        for jg in range(NJG):
                            emit_unit(ib, b, jg, get_ebt(jg))
                    emit_attnv(state["pending"])
                    state["pending"] = None
                    emit_tail_b(ib, b, po_b)
            return

        if var == "wide0":
            po0 = [psO.tile([DH + 1, IB], F32, name=f"pow0{b}", tag="po") for b in range(B)]
            po1 = [psO.tile([DH + 1, IB], F32, name=f"pow1{b}", tag="po") for b in range(B)]
            for ibk in range(NIB):
                emit_proj_chunk(ibk)
                emit_jg2(0, 2 * ibk, po0, active=0)
                emit_jg2(0, 2 * ibk + 1, po0, active=0)
                if ibk >= 1:
                    emit_jg2(1, ibk - 1, po1, active=0)
            flush_fifo(0)
            for b in range(B):
                emit_tail_b(0, b, po0[b])
            for jg in (3, 4, 5, 6, 7):
                emit_jg2(1, jg, po1, active=1)
            flush_fifo(1)
            for b in range(B):
                emit_tail_b(1, b, po1[b])
            rest = (2, 3)
        else:
            rest = tuple(range(NIB))
        for ib in rest:
            if var in ("b16p", "pair", "pair2", "pair3", "pair4", "pair5"):
                poA = psO.tile([128, IB], F32, name="poA", tag="po")
                poB = psO.tile([128, IB], F32, name="poB", tag="po")
                po = [(poA, 0), (poA, 64), (poB, 0), (poB, 64)]
            else:
                po = [psO.tile([DH + 1, IB], F32, name=f"po{b}", tag="po") for b in range(B)]
            if ib == 0:
                # interleave projection chunks with the first i-block's sweep:
                # attention(ib=0, jg) only needs k/v projection chunk jg//2
                ejg = emit_jg_esplit if var == "esplit" else emit_jg
                for ibk in range(NIB):
                    emit_proj_chunk(ibk)
                    ejg(0, 2 * ibk)
                    ejg(0, 2 * ibk + 1)
            else:
                for jg in range(NJG):
                    (emit_jg_esplit if var == "esplit" else emit_jg)(ib, jg)
            flush_fifo(ib)
            if state["pending"] is not None:
                emit_attnv(state["pending"])
            state["pending"] = None
            while state["hpend"]:
                emit_attnv_half(state["hpend"].pop(0))
            if var in ("pair4", "pair5"):
                # round-robin the tail stages across batches: all DVE po->SBUF
                # copies first (frees po banks early), then all transposes +
                # reciprocals, then out-proj matmuls interleaved with their
                # scale/store — DVE and PE pipeline instead of ping-ponging
                infos = []
                for b in range(B):
                    prow = po[b][1]
                    ot = outp.tile([128, IB], OTD, name="ot4", tag="ot4", bufs=6)
                    nc.vector.tensor_copy(ot[prow : prow + DH + 1, :], po_ap(po[b]))
                    drow = prow + DH
                    dn_t = ot
                    if drow > 64:
                        dns = outp.tile([1, IB], OTD, name="dns4", tag="dns4", bufs=3)
                        nc.sync.dma_start(dns[0:1, :], ot[drow : drow + 1, :])
                        dn_t, drow = dns, 0
                    infos.append((b, prow, ot, dn_t, drow))
                rts = []
                for b, prow, ot, dn_t, drow in infos:
                    zt = psO.tile([128, ZS * NIB], OTD, name="zt", tag="po")
                    for s in range(IB // JC):
                        nc.tensor.matmul(
                            zt[:, ZS * s : ZS * s + 1],
                            dn_t[drow : drow + 1, s * JC : (s + 1) * JC],
                            idt[drow : drow + 1, :],
                            is_transpose=True,
                            skip_group_check=True,
                        )
                    rt = outp.tile([128, ZS * NIB], F32, name="rt4", tag="rt4", bufs=6)
                    nc.vector.reciprocal(rt[:], zt[:])
                    rts.append(rt)
                for s in range(IB // JC):
                    for (b, prow, ot, dn_t, drow), rt in zip(infos, rts):
                        pw = psO.tile([128, D], F32, name="pw", tag="po")
                        nc.tensor.matmul(pw[:], ot[prow : prow + DH, s * JC : (s + 1) * JC],
                                         wo_sb[prow : prow + DH, :], start=True, stop=True)
                        ft = outp.tile([128, D], OTD, name="ft")
                        nc.vector.tensor_scalar_mul(ft[:], pw[:], rt[:, ZS * s : ZS * s + 1])
                        r0 = ib * IB + s * JC
                        nc.sync.dma_start(out[b, r0 : r0 + JC, :], ft[:])
                continue
            for b in range(B):
                prow = po[b][1] if isinstance(po[b], tuple) else 0
                ot = outp.tile([128, IB] if var in ("b16p", "pair", "pair2", "pair3") else [DH + 1, IB], OTD, name="ot")
                if var == "otact":
                    nc.scalar.copy(ot[:], po[b][:])
                else:
                    nc.vector.tensor_copy(ot[prow : prow + DH + 1, :], po_ap(po[b]))
                drow = prow + DH
                dn_t = ot
                if drow > 64:
                    dns = outp.tile([1, IB], OTD, name="dns")
                    nc.sync.dma_start(dns[0:1, :], ot[drow : drow + 1, :])
                    dn_t, drow = dns, 0
                zt = psO.tile([128, ZS * NIB], OTD, name="zt", tag="po")
                for s in range(IB // JC):
                    nc.tensor.matmul(
                        zt[:, ZS * s : ZS * s + 1],
                        dn_t[drow : drow + 1, s * JC : (s + 1) * JC],
                        idt[drow : drow + 1, :],
                        is_transpose=True,
                        skip_group_check=True,
                    )
                rt = outp.tile([128, ZS * NIB], F32, name="rt")
                nc.vector.reciprocal(rt[:], zt[:])
                for s in range(IB // JC):
                    pw = psO.tile([128, D], F32, name="pw", tag="po")
                    nc.tensor.matmul(pw[:], ot[prow : prow + DH, s * JC : (s + 1) * JC],
                                     wo_sb[prow : prow + DH, :] if var in ("b16p", "pair", "pair2", "pair3") else wo_sb[:],
                                     start=True, stop=True)
                    ft = outp.tile([128, D], OTD, name="ft")
                    if var == "ftact" and s % 2 == 0:
                        nc.scalar.mul(ft[:], pw[:], rt[:, ZS * s : ZS * s + 1])
                    else:
                        nc.vector.tensor_scalar_mul(ft[:], pw[:], rt[:, ZS * s : ZS * s + 1])
                    r0 = ib * IB + s * JC
                    nc.sync.dma_start(out[b, r0 : r0 + JC, :], ft[:])


def _emit_body_v2(nc, tc, qT, kT, vT, wq, wk, wv, wo, eb, out):
    """v2: PE-dense schedule.

    - one head per core, batch-outer-free ib sweep as base, but:
    - po accumulators packed 2-per-PSUM-bank (partitions 0-32 / 64-96)
    - output-projection tail deferred into the next i-block's instruction
      stream so the PE never stalls on the DVE/Act tail chain
    - es/wt/ebt fused across jg pairs: one DVE multiply per (b, jgp) of
      [128, 4, 512] (DVE tensor_tensor is capped at 2x mode; bigger ops
      amortize the fixed access latency)
    - out-projection matmuls in bf16 (fp32 matmul is 4 cycles/row)
    - attn@v lags 3 units behind sim via fifo; ebt prefetched 1 pair ahead
    """
    QKD = BF16
    NJP = NJC // 4          # jg-pairs per i-block: 4 (each covers 4 jc)
    with ExitStack() as ctx:
        consts = ctx.enter_context(tc.tile_pool(name="consts", bufs=1))
        persist = ctx.enter_context(tc.tile_pool(name="persist", bufs=1))
        qkv_pool = ctx.enter_context(tc.tile_pool(name="qkv", bufs=6))
        ebt_pool = ctx.enter_context(tc.tile_pool(name="ebt", bufs=3))
        es_pool = ctx.enter_context(tc.tile_pool(name="es", bufs=3))
        wt_pool = ctx.enter_context(tc.tile_pool(name="wt", bufs=8))
        tail_pool = ctx.enter_context(tc.tile_pool(name="tail", bufs=8))
        outp = ctx.enter_context(tc.tile_pool(name="outp", bufs=8))
        psS = ctx.enter_context(tc.tile_pool(name="psS", bufs=3, space="PSUM"))
        psP = ctx.enter_context(tc.tile_pool(name="psP", bufs=2, space="PSUM"))

        wq_sb = consts.tile([128, 2, JP * DH], QKD)
        nc.sync.dma_start(wq_sb[:], wq[:, :, :])
        wk_sb = consts.tile([128, 2, JP * DH], QKD)
        nc.sync.dma_start(wk_sb[:], wk[:, :, :])
        wv_sb = consts.tile([128, 2, DH], QKD)
        nc.sync.dma_start(wv_sb[:], wv[:, :, :])
        # wo replicated on partitions 0-31 and 64-95 (odd-b out-proj reads
        # its stationary from partitions 64+ to match the packed po row)
        wo_sb = consts.tile([128, D], BF16)
        nc.sync.dma_start(wo_sb[:], wo[:, :])
        ident = consts.tile([128, 1], F32)
        nc.vector.memset(ident[:], 1.0)

        qh = [persist.tile([JP * DH, N], QKD, name=f"qh{b}") for b in range(B)]
        kh = [persist.tile([JP * DH, N], QKD, name=f"kh{b}") for b in range(B)]
        vh = [persist.tile([128, NJC, DH + 1], BF16, name=f"vh{b}") for b in range(B)]
        for b in range(B):
            nc.vector.memset(vh[b][:, :, DH : DH + 1], 1.0)

        def emit_proj_b(ib, b):
            isl = slice(ib * IB, (ib + 1) * IB)
            qt = qkv_pool.tile([128, 2, IB], QKD, name="qt")
            nc.sync.dma_start(qt[:], qT[b, ib])
            kt = qkv_pool.tile([128, 2, IB], QKD, name="kt")
            nc.sync.dma_start(kt[:], kT[b, ib])
            # q in PE columns 0-63, k in 64-127: the two streams run
            # concurrently on separate XBUSes
            psqk = psS.tile([128, IB], F32, name="psqk", tag="s")
            nc.tensor.matmul(psqk[0:64, :], wq_sb[:, 0, :], qt[:, 0, :],
                             start=True, stop=False, skip_group_check=True)
            nc.tensor.matmul(psqk[64:128, :], wk_sb[:, 0, :], kt[:, 0, :],
                             start=True, stop=False, skip_group_check=True)
            nc.tensor.matmul(psqk[0:64, :], wq_sb[:, 1, :], qt[:, 1, :],
                             start=False, stop=True, skip_group_check=True)
            nc.tensor.matmul(psqk[64:128, :], wk_sb[:, 1, :], kt[:, 1, :],
                             start=False, stop=True, skip_group_check=True)
            nc.vector.tensor_copy(qh[b][:, isl], psqk[0:64, :])
            nc.vector.tensor_copy(kh[b][:, isl], psqk[64:128, :])

            vt = qkv_pool.tile([128, 2, IB], QKD, name="vt")
            nc.sync.dma_start(vt[:], vT[b, ib])
            psv = psS.tile([128, IB // JC, DH], F32, name="psv", tag="s")
            for jl in range(IB // JC):
                jsl = slice(jl * JC, (jl + 1) * JC)
                nc.tensor.matmul(psv[:, jl, :], vt[:, 0, jsl], wv_sb[:, 0, :],
                                 start=True, stop=False, skip_group_check=True)
                nc.tensor.matmul(psv[:, jl, :], vt[:, 1, jsl], wv_sb[:, 1, :],
                                 start=False, stop=True, skip_group_check=True)
            nc.vector.tensor_copy(vh[b][:, ib * (IB // JC) : (ib + 1) * (IB // JC), 0:DH], psv[:])

        # ---- attention ----
        ebts = {}

        def get_ebt(ib, jgp):
            if (ib, jgp) not in ebts:
                t4 = ebt_pool.tile([128, 4, IB], BF16, name="ebt4")
                nc.sync.dma_start(t4[:, 0:2, :], eb[ib, 2 * jgp])
                nc.sync.dma_start(t4[:, 2:4, :], eb[ib, 2 * jgp + 1])
                ebts[(ib, jgp)] = t4
            return ebts[(ib, jgp)]

        fifo = []
        LAG = 4
        pending = []  # deferred tail closures

        def attnv_emit(e):
            po_t, prow, b, jgp, wt4 = e
            for t in range(4):
                jc = jgp * 4 + t
                nc.tensor.matmul(
                    po_t[prow : prow + DH + 1, :], vh[b][:, jc, :], wt4[:, t, :],
                    start=(jc == 0), stop=(jc == NJC - 1),
                    skip_group_check=True,
                )

        def attnv_emit_pair(e0, e1):
            # interleave the two column-groups (cols 0-63 / 64-127) so the PE
            # can stream both via separate XBUSes concurrently
            po0, prow0, b0, jgp0, wt0 = e0
            po1, prow1, b1, jgp1, wt1 = e1
            if po0 is not po1 or prow0 == prow1 or jgp0 != jgp1:
                attnv_emit(e0)
                attnv_emit(e1)
                return
            for t in range(4):
                jc = jgp0 * 4 + t
                nc.tensor.matmul(
                    po0[prow0 : prow0 + DH + 1, :], vh[b0][:, jc, :], wt0[:, t, :],
                    start=(jc == 0), stop=(jc == NJC - 1),
                    skip_group_check=True,
                )
                nc.tensor.matmul(
                    po1[prow1 : prow1 + DH + 1, :], vh[b1][:, jc, :], wt1[:, t, :],
                    start=(jc == 0), stop=(jc == NJC - 1),
                    skip_group_check=True,
                )

        def emit_unit(ib, jgp, b, po_t, prow):
            isl = slice(ib * IB, (ib + 1) * IB)
            ebt4 = get_ebt(ib, jgp)
            es4 = es_pool.tile([128, 4, IB], BF16, name="es4")
            for half in range(2):
                jg = 2 * jgp + half
                ps = psS.tile([128, JP, IB], F32, name="ps", tag="s")
                for t in range(JP):
                    jc = jg * JP + t
                    psl = slice(32 * t, 32 * (t + 1))
                    nc.tensor.matmul(
                        ps[:, t, :],
                        kh[b][psl, jc * JC : (jc + 1) * JC],
                        qh[b][psl, isl],
                        start=True, stop=True,
                    )
                nc.scalar.activation(es4[:, 2 * half : 2 * half + 2, :], ps[:], AF.Exp)
            wt4 = wt_pool.tile([128, 4, IB], BF16, name="wt4")
            nc.vector.tensor_mul(wt4[:], es4[:], ebt4[:])
            fifo.append((po_t, prow, b, jgp, wt4))
            while len(fifo) >= LAG + 2:
                attnv_emit_pair(fifo.pop(0), fifo.pop(0))

        def emit_tail_dve(po_t, prow, otp, dnp):
            # frees the po bank: values -> bf16, denominator row -> f32
            nc.vector.tensor_copy(otp[prow : prow + DH, :], po_t[prow : prow + DH, :])
            nc.vector.tensor_copy(dnp[prow + DH : prow + DH + 1, :],
                                  po_t[prow + DH : prow + DH + 1, :])

        def queue_tail(ib, b, prow, otp, dnp):
            # finely-chopped deferred tail: each closure is popped at a
            # different point of the next i-block's unit stream so PSUM WAR
            # dependencies (pw buffer vs previous ft) are long resolved
            def head():
                # PE operands must sit at base partition 0/32/64; the odd-b
                # denominator row lives at 96, so shift it down via SBUF DMA
                drow = prow + DH
                dn_t = dnp
                if drow > 64:
                    dns = tail_pool.tile([1, IB], F32, name="dns")
                    nc.sync.dma_start(dns[0:1, :], dnp[drow : drow + 1, :])
                    dn_t, drow = dns, 0
                zt = psP.tile([128, NIB], F32, name="zt", tag="pop")
                for s in range(IB // JC):
                    nc.tensor.matmul(
                        zt[:, s : s + 1],
                        dn_t[drow : drow + 1, s * JC : (s + 1) * JC],
                        ident[drow : drow + 1, :],
                        is_transpose=True,
                        skip_group_check=True,
                    )
                rt = outp.tile([128, NIB], F32, name="rt")
                nc.vector.reciprocal(rt[:], zt[:])
                state["rt"] = rt

            pending.append(head)

            def mk_body(s):
                def body():
                    rt = state["rt_map"][(ib, b)]
                    pw = psP.tile([128, D], F32, name="pw", tag="pop")
                    nc.tensor.matmul(pw[:], otp[prow : prow + DH, s * JC : (s + 1) * JC],
                                     wo_sb[prow : prow + DH, :], start=True, stop=True)
                    ft = outp.tile([128, D], F32, name="ft")
                    nc.vector.tensor_scalar_mul(ft[:], pw[:], rt[:, s : s + 1])
                    r0 = ib * IB + s * JC
                    nc.sync.dma_start(out[b, r0 : r0 + JC, :], ft[:])
                return body

            def head_save():
                head()
                state["rt_map"][(ib, b)] = state["rt"]

            pending[-1] = head_save
            for s in range(IB // JC):
                pending.append(mk_body(s))

        state = {"rt": None, "rt_map": {}}

        def pop_pending():
            if pending:
                pending.pop(0)()

        for ib in range(NIB):
            po_pairs = [psP.tile([128, IB], F32, name="pop", tag="pop") for p in range(2)]
            bslot = lambda b: (po_pairs[b // 2], 64 * (b % 2))
            get_ebt(ib, 0)
            for jgp in range(NJP):
                if jgp + 1 < NJP:
                    get_ebt(ib, jgp + 1)
                elif ib + 1 < NIB:
                    get_ebt(ib + 1, 0)
                for b in range(B):
                    if ib == 0:
                        emit_proj_b(jgp, b)
                    po_t, prow = bslot(b)
                    emit_unit(ib, jgp, b, po_t, prow)
                    pop_pending()
                    pop_pending()
            # drain; free each po pair's bank as soon as its last attnv is out
            otps = [tail_pool.tile([128, IB], BF16, name=f"otp{p}") for p in range(2)]
            dnps = [tail_pool.tile([128, IB], F32, name=f"dnp{p}") for p in range(2)]
            done_pairs = set()

            def free_pair(p):
                if p in done_pairs:
                    return
                done_pairs.add(p)
                for b in (2 * p, 2 * p + 1):
                    po_t, prow = bslot(b)
                    emit_tail_dve(po_t, prow, otps[p], dnps[p])
                for b in (2 * p, 2 * p + 1):
                    _, prow = bslot(b)
                    queue_tail(ib, b, prow, otps[p], dnps[p])

            while len(fifo) >= 2:
                e0, e1 = fifo.pop(0), fifo.pop(0)
                attnv_emit_pair(e0, e1)
                if e0[3] == NJP - 1:  # last jg-pair of this i-block
                    free_pair(0 if e0[2] < 2 else 1)
            while fifo:
                attnv_emit(fifo.pop(0))
            for p in range(2):
                free_pair(p)
        while pending:
            pop_pending()


_CACHE = {}


def _get_nc(reps=1, var=None):
    if var is None:
        var = VAR
    key = ("nc", reps, var)
    if key not in _CACHE:
        QKD = BF16 if QK_BF16 else F32
        nc = bacc.Bacc("TRN2", target_bir_lowering=False, debug=False, num_devices=NCORES)
        qT = nc.dram_tensor("qT", [B, NIB, 128, 2, IB], QKD, kind="ExternalInput")
        kT = nc.dram_tensor("kT", [B, NIB, 128, 2, IB], QKD, kind="ExternalInput")
        vT = nc.dram_tensor("vT", [B, NIB, 128, 2, IB], QKD, kind="ExternalInput")
        wq = nc.dram_tensor("wq", [128, 2, JP * DH], QKD, kind="ExternalInput")
        wk = nc.dram_tensor("wk", [128, 2, JP * DH], QKD, kind="ExternalInput")
        wv = nc.dram_tensor("wv", [128, 2, DH], QKD, kind="ExternalInput")
        if var.startswith("v2"):
            wo = nc.dram_tensor("wo", [128, D], BF16, kind="ExternalInput")
        elif var == "b16p":
            wo = nc.dram_tensor("wo", [128, D], BF16, kind="ExternalInput")
        elif var in ("pair", "pair2", "pair3", "pair4", "pair5"):
            wo = nc.dram_tensor("wo", [128, D], F32, kind="ExternalInput")
        elif var == "b16":
            wo = nc.dram_tensor("wo", [DH, D], BF16, kind="ExternalInput")
        else:
            wo = nc.dram_tensor("wo", [DH, D], F32, kind="ExternalInput")
        eb = nc.dram_tensor("eb", [NIB, NJG, 128, JP, IB], BF16, kind="ExternalInput")
        out = nc.dram_tensor("out", [B, N, D],
                             BF16 if var in ("b16", "b16p") else F32, kind="ExternalOutput")
        build_kernel(
            nc,
            qT.ap(), kT.ap(), vT.ap(),
            wq.ap(), wk.ap(), wv.ap(), wo.ap(),
            eb.ap(), out.ap(),
            reps=reps, var=var,
        )
        nc.compile()
        _CACHE[key] = nc
    return _CACHE[key]


def _np_dtype():
    return ml_dtypes.bfloat16 if QK_BF16 else np.float32


def _dn_layout(x):
    """[B, N, D] -> [B, NIB, 128, 2, IB]; tile (b, ib)[p, c, col] = x[b, ib*IB+col, c*128+p]."""
    t = x.reshape(B, NIB, IB, 2, 128)
    return np.ascontiguousarray(t.transpose(0, 1, 4, 3, 2).astype(_np_dtype()))


def _w_layout(w, rep):
    """[32, 256] (out, in) -> [128, 2, rep*32] transposed, M-replicated."""
    wt = np.ascontiguousarray(w.T)                       # [256, 32]
    wt = np.concatenate([wt] * rep, axis=1)              # [256, rep*32]
    return np.ascontiguousarray(
        wt.reshape(2, 128, rep * DH).transpose(1, 0, 2).astype(_np_dtype())
    )


def _eb_layout(pb_h):
    """[N, N] pos_bias head -> [NIB, NJG, 128, JP, IB] tiled exp-bias (bf16).
    tile (ib, jg)[p, t, col] = exp(pb_h[ib*IB+col, jg*(JP*128)+t*128+p])."""
    e = np.exp(pb_h)                                     # [i, j]
    x = e.reshape(NIB, IB, NJG, JP, 128)                 # [ib, col, jg, t, p]
    return np.ascontiguousarray(x.transpose(0, 2, 4, 3, 1).astype(ml_dtypes.bfloat16))


def make_in_maps(q, k, v, pos_bias, Wq, Wk, Wv, Wo):
    q = np.asarray(q, dtype=np.float32)
    k = np.asarray(k, dtype=np.float32)
    v = np.asarray(v, dtype=np.float32)
    pos_bias = np.asarray(pos_bias, dtype=np.float32)
    Wq = np.asarray(Wq, dtype=np.float32)
    Wk = np.asarray(Wk, dtype=np.float32)
    Wv = np.asarray(Wv, dtype=np.float32)
    Wo = np.asarray(Wo, dtype=np.float32)

    qT = _dn_layout(q)
    kT = _dn_layout(k)
    vT = _dn_layout(v)

    in_maps = []
    for h in range(NCORES):
        hs = slice(h * DH, (h + 1) * DH)
        if VAR.startswith("v2") or VAR == "b16p":
            wo_h = np.zeros((128, D), dtype=ml_dtypes.bfloat16)
            wo_h[0:DH, :] = Wo[:, hs].T.astype(ml_dtypes.bfloat16)
            wo_h[64 : 64 + DH, :] = wo_h[0:DH, :]
        elif VAR in ("pair", "pair2", "pair3", "pair4", "pair5"):
            wo_h = np.zeros((128, D), dtype=np.float32)
            wo_h[0:DH, :] = Wo[:, hs].T
            wo_h[64 : 64 + DH, :] = wo_h[0:DH, :]
        elif VAR == "b16":
            wo_h = np.ascontiguousarray(Wo[:, hs].T.astype(ml_dtypes.bfloat16))
        else:
            wo_h = np.ascontiguousarray(Wo[:, hs].T)
        in_maps.append({
            "qT": qT,
            "kT": kT,
            "vT": vT,
            "wq": _w_layout(SCALE * Wq[hs, :], JP),
            "wk": _w_layout(Wk[hs, :], JP),
            "wv": _w_layout(Wv[hs, :], 1),
            "wo": wo_h,
            "eb": _eb_layout(pos_bias[h]),
        })
    return in_maps


def kernel(q, k, v, pos_bias, Wq, Wk, Wv, Wo):
    nc = _get_nc()
    in_maps = make_in_maps(q, k, v, pos_bias, Wq, Wk, Wv, Wo)
    res = run_bass_kernel_spmd(nc, in_maps, core_ids=list(range(NCORES)))
    acc = res.results[0]["out"].astype(np.float32)
    for c in range(1, NCORES):
        acc = acc + res.results[c]["out"]
    return acc

